# revision 3
# baseline (speedup 1.0000x reference)
"""Distributed blocked-cumprod kernel for Trainium2 (8 NeuronCores).

Problem: alpha_bars = cumprod(1 - betas) over T = 2**25 f32 elements.

Strategy (classic parallel-scan decomposition, 3 levels):
  - Shard T contiguously across 8 cores (4,194,304 elements each).
  - Per core, view the shard as [128 partitions x 32768 cols] row-major
    (partition p owns a contiguous 32768-element block).
  - Phase 1: stream 16 column-tiles [128 x 2048]; ScalarE computes
    alpha = 1 - beta in place; VectorE runs a chained tensor_tensor_scan
    (op0=mult) along the free dim -> per-partition local cumprods.
  - Phase 2: row totals = last column; transpose via PE matmul with
    identity -> [1,128]; scan -> inclusive row prefixes; exclusive
    shift; core total -> AllGather (8 scalars) -> exclusive cross-core
    prefix selected with a per-core onehot input; fold the core prefix
    into the transpose-back matmul -> full per-partition prefix [128,1].
  - Phase 3: VectorE tensor_scalar multiplies each tile by the
    per-partition prefix in place; DMA out.
"""

import sys

sys.path.insert(0, "/opt/trn_rl_repo")

import numpy as np

from concourse import bacc, mybir, tile
from concourse.bass_utils import run_bass_kernel_spmd

NCORES = 8
P = 128
T_FULL = 33554432
TILE_COLS = 2048

_F32 = mybir.dt.float32


def build_nc(shard_len: int, tile_cols: int):
    cols = shard_len // P
    ntiles = cols // tile_cols
    assert shard_len == P * cols and cols % tile_cols == 0

    mult = mybir.AluOpType.mult
    bypass = mybir.AluOpType.bypass
    add = mybir.AluOpType.add
    identity = mybir.ActivationFunctionType.Identity

    nc = bacc.Bacc(num_devices=NCORES)
    betas = nc.declare_dram_parameter("betas", [shard_len], _F32, isOutput=False)
    eye_in = nc.declare_dram_parameter("eye", [P, P], _F32, isOutput=False)
    onehot_in = nc.declare_dram_parameter("onehot", [1, NCORES], _F32, isOutput=False)
    out = nc.declare_dram_parameter("out", [shard_len], _F32, isOutput=True)

    bview = betas.ap().rearrange("(p c) -> p c", p=P)
    oview = out.ap().rearrange("(p c) -> p c", p=P)

    with tile.TileContext(nc) as tc:
        with (
            tc.tile_pool(name="data", bufs=1) as data_pool,
            tc.tile_pool(name="small", bufs=1) as small_pool,
            tc.tile_pool(name="psum", bufs=1, space="PSUM") as psum_pool,
            tc.tile_pool(name="dram", bufs=1, space="DRAM") as dram_pool,
        ):
            A = [
                data_pool.tile([P, tile_cols], _F32, name=f"a{j}")
                for j in range(ntiles)
            ]
            eye_sb = small_pool.tile([P, P], _F32, name="eye_sb")
            nc.sync.dma_start(eye_sb[:], eye_in[:, :])
            oh_sb = small_pool.tile([1, NCORES], _F32, name="oh_sb")
            nc.sync.dma_start(oh_sb[:], onehot_in[:, :])

            # Phase 1: load, alpha = 1 - beta, chained local scan.
            for j in range(ntiles):
                nc.sync.dma_start(A[j][:], bview[:, j * tile_cols : (j + 1) * tile_cols])
            for j in range(ntiles):
                nc.scalar.activation(A[j][:], A[j][:], identity, bias=1.0, scale=-1.0)
            for j in range(ntiles):
                init = 1.0 if j == 0 else A[j - 1][:, tile_cols - 1 : tile_cols]
                nc.vector.tensor_tensor_scan(A[j][:], A[j][:], A[j][:], init, mult, bypass)

            # Phase 2: prefixes.  Row totals -> [1,128] via PE transpose.
            rowt_ps = psum_pool.tile([1, P], _F32, name="rowt_ps")
            nc.tensor.matmul(
                rowt_ps[:],
                A[ntiles - 1][:, tile_cols - 1 : tile_cols],
                eye_sb[:],
                start=True,
                stop=True,
            )
            rowt = small_pool.tile([1, P], _F32, name="rowt")
            nc.scalar.copy(rowt[:], rowt_ps[:])
            rowi = small_pool.tile([1, P], _F32, name="rowi")
            nc.vector.tensor_tensor_scan(rowi[:], rowt[:], rowt[:], 1.0, mult, bypass)
            exc = small_pool.tile([1, P], _F32, name="exc")
            nc.vector.memset(exc[0:1, 0:1], 1.0)
            nc.vector.tensor_copy(exc[0:1, 1:P], rowi[0:1, 0 : P - 1])

            # Core total = rowi[0, P-1]; exchange across the chip.
            cc_in = dram_pool.tile([1, 1], _F32, name="cc_in")
            cc_out = dram_pool.tile([NCORES, 1], _F32, name="cc_out", addr_space="Shared")
            nc.sync.dma_start(cc_in[:], rowi[0:1, P - 1 : P])
            nc.gpsimd.collective_compute(
                "AllGather",
                bypass,
                replica_groups=[list(range(NCORES))],
                ins=[cc_in.opt()],
                outs=[cc_out.opt()],
            )
            gath = small_pool.tile([1, NCORES], _F32, name="gath")
            nc.sync.dma_start(gath[:], cc_out[:, 0:1])
            gi = small_pool.tile([1, NCORES], _F32, name="gi")
            nc.vector.tensor_tensor_scan(gi[:], gath[:], gath[:], 1.0, mult, bypass)
            ge = small_pool.tile([1, NCORES], _F32, name="ge")
            nc.vector.memset(ge[0:1, 0:1], 1.0)
            nc.vector.tensor_copy(ge[0:1, 1:NCORES], gi[0:1, 0 : NCORES - 1])
            sel = small_pool.tile([1, NCORES], _F32, name="sel")
            nc.vector.tensor_tensor(sel[:], ge[:], oh_sb[:], mult)
            cpref = small_pool.tile([1, 1], _F32, name="cpref")
            nc.vector.tensor_reduce(cpref[:], sel[:], mybir.AxisListType.X, add)

            # full_prefix[p] = exc[0,p] * cpref  (transpose-back + broadcast
            # multiply in one K=1 matmul).
            fp_ps = psum_pool.tile([P, 1], _F32, name="fp_ps")
            nc.tensor.matmul(fp_ps[:], exc[:], cpref[:], start=True, stop=True)
            fprefix = small_pool.tile([P, 1], _F32, name="fprefix")
            nc.scalar.copy(fprefix[:], fp_ps[:])

            # Phase 3: scale in place, store.
            for j in range(ntiles):
                nc.vector.tensor_scalar_mul(A[j][:], A[j][:], fprefix[:])
                nc.sync.dma_start(oview[:, j * tile_cols : (j + 1) * tile_cols], A[j][:])

    nc.compile()
    return nc


def make_in_maps(betas: np.ndarray):
    shard = betas.size // NCORES
    eye = np.eye(P, dtype=np.float32)
    in_maps = []
    for k in range(NCORES):
        onehot = np.zeros((1, NCORES), dtype=np.float32)
        onehot[0, k] = 1.0
        in_maps.append(
            {
                "betas": np.ascontiguousarray(betas[k * shard : (k + 1) * shard]),
                "eye": eye,
                "onehot": onehot,
            }
        )
    return in_maps


def kernel(betas: np.ndarray) -> np.ndarray:
    betas = np.asarray(betas, dtype=np.float32).reshape(-1)
    assert betas.size == T_FULL, betas.size
    nc = build_nc(T_FULL // NCORES, TILE_COLS)
    in_maps = make_in_maps(betas)
    res = run_bass_kernel_spmd(nc, in_maps, core_ids=list(range(NCORES)))
    return np.concatenate([res.results[k]["out"] for k in range(NCORES)])
